# revision 22
# baseline (speedup 1.0000x reference)
# Bass/Tile kernel for nn_Decoder: 4-layer dense transformer, B=2 L=2048 D=1024 H=16 V=32000.
#
# Sharding: token-parallel over 8 cores with INTERLEAVED query ownership —
# core c (batch b=c//4, cb=c%4) owns query tiles {4j+cb : j=0..3} of 128 tokens
# each, so causal work per core is balanced and the SPMD program can skip
# key tiles by column-suffix uniformly. Per-layer K/V AllGather within batch
# groups [[0-3],[4-7]]; vocab-sharded logits after an 8-rank AllGather of x.
#
# Layout: activations as xT [d(part-tiled), t]; matmuls bf16 with fp32 PSUM
# accumulate; weights host-pretiled so every DMA has >=1KB contiguous runs.
# V is carried as [t, h, 65] with column 64 = 1.0 (ones column produced by an
# extra e0 @ vpat matmul) so the softmax denominator falls out of the AV
# matmul. Causality: scores for key tile kt are computed only on query columns
# [128*(kt//4) : 512]; the first 128 columns of that suffix are the diagonal
# band and get masked (one paired DVE multiply per two key tiles) with
# SBUF-resident [128,128] 0/1 patterns (all-ones / triangle / all-zero by
# kt%4 vs cb). The AV accumulation runs kt descending so the first emitted
# matmul (start=True) clears the bank and the last (widest) carries stop.
import contextlib
import math

import numpy as np

import concourse.mybir as mybir
import concourse.tile as tile
from concourse import bacc

P = 128
D = 1024
H = 16
DH = 64
HG = 4            # heads per V-projection group (4*65 = 260 <= 512 psum cols)
FF = 2048
L = 2048
B = 2
V = 32000
NCORE = 8
T = 512           # own tokens per core
KO = D // P       # 8
FKO = FF // P     # 16
NKT = (4 * T) // P  # 16 key tiles (full batch)
NJ = T // P       # 4 query column blocks per core
VS = V // NCORE   # 4000
VN = 500          # vocab N-tile width
VT = VS // VN     # 8
EPS = 1e-6
f32 = mybir.dt.float32
bf16 = mybir.dt.bfloat16
AF = mybir.ActivationFunctionType
ALU = mybir.AluOpType
VROW = H * 65     # 1040


def build(n_layers=4, do_logits=True, nz_bias=False, nz_blog=False,
          want_xout=False, sim_local=False):
    nc = bacc.Bacc(None, target_bir_lowering=False)

    x0T = nc.dram_tensor("x0T", [P, KO, T], f32, kind="ExternalInput")
    wq = nc.dram_tensor("wq", [n_layers, P, KO, D], bf16, kind="ExternalInput")
    wk = nc.dram_tensor("wk", [n_layers, P, KO, D], bf16, kind="ExternalInput")
    wv = nc.dram_tensor("wv", [n_layers, P, KO, VROW], bf16, kind="ExternalInput")
    wo = nc.dram_tensor("wo", [n_layers, P, KO, D], bf16, kind="ExternalInput")
    wproj = nc.dram_tensor("wproj", [n_layers, P, KO, FF], bf16, kind="ExternalInput")
    wup = nc.dram_tensor("wup", [n_layers, P, KO, FF], bf16, kind="ExternalInput")
    wdown = nc.dram_tensor("wdown", [n_layers, P, FKO, D], bf16, kind="ExternalInput")
    vpat = nc.dram_tensor("vpat", [n_layers, P, VROW], bf16, kind="ExternalInput")
    maskpat = nc.dram_tensor("maskpat", [P, NJ, P], bf16, kind="ExternalInput")
    ones_d = nc.dram_tensor("ones_d", [P, P], bf16, kind="ExternalInput")
    e0_d = nc.dram_tensor("e0_d", [P, P], bf16, kind="ExternalInput")
    # bias columns: [bq 0:8 | bk 8:16 | bo 16:24 | bproj 24:40 | bup 40:56 | bdown 56:64]
    ball = (nc.dram_tensor("ball", [n_layers, P, 64], f32, kind="ExternalInput")
            if nz_bias else None)
    if do_logits:
        wlog = nc.dram_tensor("wlog", [VT, P, KO, VN], bf16, kind="ExternalInput")
        blogpat = (nc.dram_tensor("blogpat", [P, VS], bf16, kind="ExternalInput")
                   if nz_blog else None)
        logits = nc.dram_tensor("logits", [B * L, VS], bf16, kind="ExternalOutput")
    if want_xout:
        xout = nc.dram_tensor("xout", [P, KO, T], f32, kind="ExternalOutput")

    with tile.TileContext(nc) as tc, contextlib.ExitStack() as ctx:
        cn = ctx.enter_context(tc.tile_pool(name="cn", bufs=1))
        pb = ctx.enter_context(tc.tile_pool(name="pb", bufs=1))
        evn = ctx.enter_context(tc.tile_pool(name="evn", bufs=2))
        sm = ctx.enter_context(tc.tile_pool(name="sm", bufs=1))
        x2p = ctx.enter_context(tc.tile_pool(name="x2p", bufs=2))
        ps = ctx.enter_context(tc.tile_pool(name="ps", bufs=2, space="PSUM"))
        pa = ctx.enter_context(tc.tile_pool(name="pa", bufs=2, space="PSUM"))
        pav = ctx.enter_context(tc.tile_pool(name="pav", bufs=2, space="PSUM"))
        dr = ctx.enter_context(tc.tile_pool(name="dram", bufs=2, space="DRAM"))

        # ---- constants ----
        ones = cn.tile([P, P], bf16, tag="ones")
        nc.sync.dma_start(ones[:], ones_d[:])
        e0 = cn.tile([P, P], bf16, tag="e0")
        nc.sync.dma_start(e0[:], e0_d[:])
        masks = cn.tile([P, NJ, P], bf16, tag="masks")
        nc.sync.dma_start(masks[:], maskpat[:])
        sc_rms = cn.tile([P, 1], f32, tag="sc_rms")
        nc.any.memset(sc_rms[:], 1.0 / D)
        eps_t = cn.tile([P, 1], f32, tag="eps")
        nc.any.memset(eps_t[:], EPS)
        sc_att = cn.tile([P, 1], f32, tag="sc_att")
        nc.any.memset(sc_att[:], 1.0 / math.sqrt(DH))


        def act(dst, src_, func, bias=None, scale=None):
            kw = {}
            if bias is not None:
                kw["bias"] = bias
            if scale is not None:
                kw["scale"] = scale
            nc.scalar.activation(dst, src_, func, **kw)

        # ---- persistent activations ----
        xT = pb.tile([P, KO, T], f32, tag="xT")
        QT = pb.tile([P, KO, T], bf16, tag="QT")

        # ---- Phase 0: load xT (host provides embed+pe, transposed) ----
        nc.sync.dma_start(xT[:], x0T[:])

        fuse_xr = do_logits and not want_xout and n_layers > 0
        agx_in = agxv = None
        if do_logits:
            agx_in = dr.tile([D * T], bf16, tag="agx_in")
            agxv = agx_in.rearrange("(ko p t) -> p ko t", p=P, t=T)
        holder = {}

        def rmsnorm(nm):
            ssq = ps.tile([P, T], f32, tag="mm512")
            for ko in range(KO):
                x2 = x2p.tile([P, T], bf16, tag="x2")
                nc.vector.tensor_tensor(x2[:], xT[:, ko], xT[:, ko], ALU.mult)
                nc.tensor.matmul(ssq[:], ones[:], x2[:], start=(ko == 0),
                                 stop=(ko == KO - 1))
            srt = sm.tile([P, T], f32, tag="srt", name=f"srt_{nm}")
            nc.scalar.activation(srt[:], ssq[:], AF.Sqrt, bias=eps_t[:], scale=sc_rms[:])
            inv = sm.tile([P, T], f32, tag="inv", name=f"inv_{nm}")
            nc.vector.reciprocal(inv[:], srt[:])
            normT = pb.tile([P, KO, T], bf16, tag="nta", name=f"normT_{nm}")
            for ko in range(KO):
                nc.vector.tensor_tensor(normT[:, ko], xT[:, ko], inv[:], ALU.mult)
            return normT

        with tc.tile_pool(name="wp", bufs=2) as wp, \
             tc.tile_pool(name="wvp", bufs=1) as wvp, \
             tc.tile_pool(name="wf", bufs=3) as wf, \
             tc.tile_pool(name="kvp", bufs=1) as kvp, \
             tc.tile_pool(name="exp", bufs=6) as exp_, \
             tc.tile_pool(name="hp", bufs=1) as hp:

            for li in range(n_layers):
                bias_t = None
                if ball is not None:
                    bias_t = evn.tile([P, 64], f32, tag="bias", name=f"bias_{li}")
                    nc.sync.dma_start(bias_t[:], ball[li])

                def bcol(idx, bias_t=bias_t):
                    return None if bias_t is None else bias_t[:, idx:idx + 1]

                normT = rmsnorm(f"n1_{li}")

                agk_in = dr.tile([D * T], bf16, tag="agk_in")
                agk_out = dr.tile([4, D * T], bf16, tag="agk_out")
                agv_in = dr.tile([T * VROW], bf16, tag="agv_in")
                agv_out = dr.tile([4, T * VROW], bf16, tag="agv_out")
                kT_view = agk_in.rearrange("(ko p t) -> ko p t", p=P, t=T)
                v_view = agv_in.rearrange("(t m) -> t m", m=VROW)

                # ---- K projection -> ag_in ----
                wk_sb = {}
                for half in range(2):
                    wk_sb[half] = wp.tile([P, KO, D // 2], bf16, tag="wkq",
                                          name=f"wk{half}_{li}")
                    nc.sync.dma_start(wk_sb[half][:],
                                      wk[li, :, :, half * D // 2:(half + 1) * D // 2])
                for m in range(KO):
                    half, mh = m // 4, m % 4
                    pt = ps.tile([P, T], f32, tag="mm512")
                    for ko in range(KO):
                        nc.tensor.matmul(pt[:], wk_sb[half][:, ko, mh * P:(mh + 1) * P],
                                         normT[:, ko], start=(ko == 0),
                                         stop=(ko == KO - 1))
                    kev = evn.tile([P, T], bf16, tag="kev")
                    act(kev[:], pt[:], AF.Identity, bias=bcol(8 + m))
                    nc.sync.dma_start(kT_view[m], kev[:])

                if sim_local:
                    for r_ in range(4):
                        nc.sync.dma_start(agk_out[r_], agk_in[:])
                else:
                    nc.gpsimd.collective_compute(
                        "AllGather", ALU.bypass, ins=[agk_in[:]], outs=[agk_out[:]],
                        replica_groups=[[0, 1, 2, 3], [4, 5, 6, 7]])

                # ---- V projection (4-head groups, +ones/bias row) ----
                wv_sb = wvp.tile([P, KO, VROW], bf16, tag="wv", name=f"wv_{li}")
                nc.sync.dma_start(wv_sb[:], wv[li])
                vp_sb = sm.tile([P, VROW], bf16, tag="vpat", name=f"vp_{li}")
                nc.sync.dma_start(vp_sb[:], vpat[li])
                for mt in range(NJ):
                    for hg in range(H // HG):
                        c0 = hg * HG * 65
                        pt = ps.tile([P, T], f32, tag="mm512")
                        for ko in range(KO):
                            nc.tensor.matmul(pt[:, 0:HG * 65],
                                             normT[:, ko, mt * P:(mt + 1) * P],
                                             wv_sb[:, ko, c0:c0 + HG * 65],
                                             start=(ko == 0), stop=False)
                        nc.tensor.matmul(pt[:, 0:HG * 65], e0[:],
                                         vp_sb[:, c0:c0 + HG * 65],
                                         start=False, stop=True)
                        vev = evn.tile([P, HG * 65], bf16, tag="vev")
                        nc.vector.tensor_copy(vev[:], pt[:, 0:HG * 65])
                        nc.sync.dma_start(
                            v_view[mt * P:(mt + 1) * P, c0:c0 + HG * 65], vev[:])

                if sim_local:
                    for r_ in range(4):
                        nc.sync.dma_start(agv_out[r_], agv_in[:])
                else:
                    nc.gpsimd.collective_compute(
                        "AllGather", ALU.bypass, ins=[agv_in[:]], outs=[agv_out[:]],
                        replica_groups=[[0, 1, 2, 3], [4, 5, 6, 7]])

                # ---- Q projection (overlaps the V AllGather) ----
                wq_sb = {}
                for half in range(2):
                    wq_sb[half] = wp.tile([P, KO, D // 2], bf16, tag="wkq",
                                          name=f"wq{half}_{li}")
                    nc.sync.dma_start(wq_sb[half][:],
                                      wq[li, :, :, half * D // 2:(half + 1) * D // 2])
                for m in range(KO):
                    half, mh = m // 4, m % 4
                    pt = ps.tile([P, T], f32, tag="mm512")
                    for ko in range(KO):
                        nc.tensor.matmul(pt[:], wq_sb[half][:, ko, mh * P:(mh + 1) * P],
                                         normT[:, ko], start=(ko == 0),
                                         stop=(ko == KO - 1))
                    act(QT[:, m, :], pt[:], AF.Identity, bias=bcol(m))

                # ---- load gathered K/V into SBUF (slot-major physical order) ----
                # physical column r*4+i  <->  global key tile 4*i + r
                K_all = kvp.tile([P, KO, NKT * P], bf16, tag="K_all", name=f"K_{li}")
                V_all = kvp.tile([P, NKT, VROW], bf16, tag="V_all", name=f"V_{li}")
                # interleave so the first score pair's K (slots 3,2) and its
                # AV's V (slots 3,2) arrive before the rest
                for pair in ((3, 2), (1, 0)):
                    for r in pair:
                        src_k = agk_out[r].rearrange("(ko p t) -> p ko t", p=P, t=T)
                        nc.sync.dma_start(K_all[:, :, r * 4 * P:(r + 1) * 4 * P],
                                          src_k)
                    for r in pair:
                        src_v = agv_out[r].rearrange("(mt p m) -> p mt m",
                                                     p=P, m=VROW)
                        nc.sync.dma_start(V_all[:, r * 4:(r + 1) * 4, :], src_v)

                # ---- attention ----
                aoT = pb.tile([P, KO, T], bf16, tag="nta", name=f"aoT_{li}")
                for h in range(H):
                    pbase = DH * (h % 2)
                    hko = h // 2
                    av = pav.tile([DH + 1, T], f32, tag="av")
                    # kt descending by pair (same suffix width within a pair);
                    # pair order (even, odd) so the mask patterns for the pair
                    # are adjacent columns of `masks`.
                    for kp in range(NKT // 2 - 1, -1, -1):
                        kts = (2 * kp, 2 * kp + 1)
                        j0 = kts[0] // 4
                        sw = T - j0 * P
                        i0 = (2 * kp) % 4
                        sp = pa.tile([P, 2, T], f32, tag="sc")
                        for q_, kt in enumerate(kts):
                            phys = (kt % 4) * 4 + kt // 4
                            nc.tensor.matmul(
                                sp[:, q_, 0:sw],
                                K_all[pbase:pbase + DH, hko, phys * P:(phys + 1) * P],
                                QT[pbase:pbase + DH, hko, j0 * P:T],
                                start=True, stop=True)
                        ext = exp_.tile([P, 2, T], bf16, tag="ext")
                        nc.scalar.activation(ext[:, :, 0:sw], sp[:, :, 0:sw], AF.Exp,
                                             scale=sc_att[:])
                        nc.vector.tensor_tensor(ext[:, :, 0:P], ext[:, :, 0:P],
                                                masks[:, i0:i0 + 2, :], ALU.mult)
                        for q_, kt in enumerate(kts):
                            phys = (kt % 4) * 4 + kt // 4
                            nc.tensor.matmul(av[:, j0 * P:T],
                                             V_all[:, phys, h * 65:h * 65 + 65],
                                             ext[:, q_, 0:sw],
                                             start=(kt == NKT - 2), stop=(kt == 1))
                    invd = evn.tile([1, T], bf16, tag="invd")
                    with nc.allow_low_precision(reason="1/Z broadcast in bf16"):
                        nc.vector.reciprocal(invd[0:1, :], av[DH:DH + 1, :])
                    bcp = ps.tile([P, T], f32, tag="mm512")
                    nc.tensor.matmul(bcp[0:DH, :], ones[0:1, 0:DH], invd[0:1, :],
                                     start=True, stop=True)
                    invb = evn.tile([DH, T], bf16, tag="invb")
                    nc.vector.tensor_copy(invb[:], bcp[0:DH, :])
                    nc.vector.tensor_tensor(aoT[pbase:pbase + DH, hko, :],
                                            av[0:DH, :], invb[:], ALU.mult)

                # ---- O projection + residual ----
                wo_sb = {}
                for half in range(2):
                    wo_sb[half] = wp.tile([P, KO, D // 2], bf16, tag="wkq",
                                          name=f"wo{half}_{li}")
                    nc.sync.dma_start(wo_sb[half][:],
                                      wo[li, :, :, half * D // 2:(half + 1) * D // 2])
                for m in range(KO):
                    half, mh = m // 4, m % 4
                    pt = ps.tile([P, T], f32, tag="mm512")
                    for ko in range(KO):
                        nc.tensor.matmul(pt[:], wo_sb[half][:, ko, mh * P:(mh + 1) * P],
                                         aoT[:, ko], start=(ko == 0), stop=(ko == KO - 1))
                    if bias_t is not None:
                        ot = evn.tile([P, T], f32, tag="ot")
                        nc.scalar.activation(ot[:], pt[:], AF.Identity, bias=bcol(16 + m))
                        nc.vector.tensor_tensor(xT[:, m, :], xT[:, m, :], ot[:], ALU.add)
                    else:
                        nc.vector.tensor_tensor(xT[:, m, :], xT[:, m, :], pt[:], ALU.add)

                # ---- FFN ----
                normT = rmsnorm(f"n2_{li}")
                hts = hp.tile([P, FKO, T], bf16, tag="hts", name=f"hts_{li}")
                wpu_sb = {}
                for qt_ in range(4):
                    t_ = wf.tile([P, KO, FF // 4], bf16, tag="wpu",
                                 name=f"wp{qt_}_{li}")
                    nc.sync.dma_start(t_[:], wproj[li, :, :, qt_ * FF // 4:
                                                   (qt_ + 1) * FF // 4])
                    wpu_sb[("p", qt_)] = t_
                    t_ = wf.tile([P, KO, FF // 4], bf16, tag="wpu",
                                 name=f"wu{qt_}_{li}")
                    nc.sync.dma_start(t_[:], wup[li, :, :, qt_ * FF // 4:
                                                (qt_ + 1) * FF // 4])
                    wpu_sb[("u", qt_)] = t_
                for m in range(FKO):
                    half, mh = m // 4, m % 4
                    ptp = ps.tile([P, T], f32, tag="mm512")
                    for ko in range(KO):
                        nc.tensor.matmul(ptp[:],
                                         wpu_sb[("p", half)][:, ko, mh * P:(mh + 1) * P],
                                         normT[:, ko], start=(ko == 0),
                                         stop=(ko == KO - 1))
                    ptu = ps.tile([P, T], f32, tag="mm512")
                    for ko in range(KO):
                        nc.tensor.matmul(ptu[:],
                                         wpu_sb[("u", half)][:, ko, mh * P:(mh + 1) * P],
                                         normT[:, ko], start=(ko == 0),
                                         stop=(ko == KO - 1))
                    usb = evn.tile([P, T], bf16, tag="uev")
                    act(usb[:], ptu[:], AF.Identity, bias=bcol(40 + m))
                    psb = evn.tile([P, T], bf16, tag="pev")
                    if bias_t is not None:
                        psb2 = evn.tile([P, T], f32, tag="pev2")
                        nc.scalar.activation(psb2[:], ptp[:], AF.Identity,
                                             bias=bcol(24 + m))
                        nc.vector.tensor_tensor(psb[:], psb2[:], usb[:], ALU.mult)
                    else:
                        nc.vector.tensor_tensor(psb[:], ptp[:], usb[:], ALU.mult)
                    nc.scalar.activation(hts[:, m, :], psb[:], AF.Silu)
                for qt_ in range(4):
                    t_ = wf.tile([P, FKO, D // 4], bf16, tag="wpu",
                                 name=f"wd{qt_}_{li}")
                    nc.sync.dma_start(t_[:], wdown[li, :, :, qt_ * D // 4:
                                                 (qt_ + 1) * D // 4])
                    wpu_sb[("d", qt_)] = t_
                last = fuse_xr and li == n_layers - 1
                if last:
                    # final layer: residual lands directly in the bf16 x used
                    # for the logits AllGather; xT itself is dead afterwards
                    xr = pb.tile([P, KO, T], bf16, tag="QT", name="xr")
                    holder["xr"] = xr
                for m in range(KO):
                    half, mh = m // 2, m % 2
                    pt = ps.tile([P, T], f32, tag="mm512")
                    for ko in range(FKO):
                        nc.tensor.matmul(pt[:],
                                         wpu_sb[("d", half)][:, ko, mh * P:(mh + 1) * P],
                                         hts[:, ko], start=(ko == 0),
                                         stop=(ko == FKO - 1))
                    if bias_t is not None:
                        dt_ = evn.tile([P, T], f32, tag="ot")
                        nc.scalar.activation(dt_[:], pt[:], AF.Identity,
                                             bias=bcol(56 + m))
                        src_add = dt_
                    else:
                        src_add = pt
                    if last:
                        nc.vector.tensor_tensor(xr[:, m, :], xT[:, m, :],
                                                src_add[:], ALU.add)
                        nc.sync.dma_start(agxv[:, m, :], xr[:, m, :])
                    else:
                        nc.vector.tensor_tensor(xT[:, m, :], xT[:, m, :],
                                                src_add[:], ALU.add)

        if want_xout:
            nc.sync.dma_start(xout[:], xT[:])

        if do_logits:
            if "xr" not in holder:
                xr = pb.tile([P, KO, T], bf16, tag="QT")  # reuse QT slot
                nc.vector.tensor_copy(xr[:], xT[:])
                nc.sync.dma_start(agxv[:], xr[:])
            agx_out = dr.tile([NCORE, D * T], bf16, tag="agx_out",
                              addr_space="Local" if sim_local else "Shared")
            if sim_local:
                for g_ in range(NCORE):
                    nc.sync.dma_start(agx_out[g_], agx_in[:])
            else:
                nc.gpsimd.collective_compute(
                    "AllGather", ALU.bypass, ins=[agx_in[:]], outs=[agx_out[:]],
                    replica_groups=[[0, 1, 2, 3, 4, 5, 6, 7]])
            with tc.tile_pool(name="lxp", bufs=1) as lxp, \
                 tc.tile_pool(name="wlp", bufs=2) as wlp, \
                 tc.tile_pool(name="osp", bufs=3) as osp:
                # physical row tile r*4+j  <->  global row tile (r//4)*16 + 4j + r%4
                X_all = lxp.tile([P, KO, NCORE * NJ * P], bf16, tag="X_all")
                for r in range(NCORE):
                    nc.sync.dma_start(
                        X_all[:, :, r * NJ * P:(r + 1) * NJ * P],
                        agx_out[r].rearrange("(ko p t) -> p ko t", p=P, t=T))
                blt = None
                if blogpat is not None:
                    blt = lxp.tile([P, VS], bf16, tag="blt")
                    nc.sync.dma_start(blt[:], blogpat[:])
                # [B*L, VS] rows = ((b2*16 + 4j + cb) * 128 + p)
                ldst = logits.rearrange("(b2 j cb p) v -> b2 cb p j v",
                                        b2=B, j=NJ, cb=4, p=P)
                for vt in range(VT):
                    wt = wlp.tile([P, KO, VN], bf16, tag="wlog")
                    nc.sync.dma_start(wt[:], wlog[vt])
                    for r in range(NCORE):
                        osb = osp.tile([P, NJ, VN], bf16, tag="osb")
                        for j in range(NJ):
                            rp = r * NJ + j
                            pt = ps.tile([P, T], f32, tag="mm512")
                            for ko in range(KO):
                                last = (ko == KO - 1) and blt is None
                                nc.tensor.matmul(pt[:, 0:VN],
                                                 X_all[:, ko, rp * P:(rp + 1) * P],
                                                 wt[:, ko], start=(ko == 0), stop=last)
                            if blt is not None:
                                nc.tensor.matmul(pt[:, 0:VN], e0[:],
                                                 blt[:, vt * VN:(vt + 1) * VN],
                                                 start=False, stop=True)
                            nc.vector.tensor_copy(osb[:, j, :], pt[:, 0:VN])
                        nc.sync.dma_start(
                            ldst[r // 4, r % 4, :, :, vt * VN:(vt + 1) * VN], osb[:])

    nc.compile()
    return nc


def host_inputs(inp, n_layers=4, do_logits=True):
    """Build per-core in_maps from the full model inputs dict (numpy)."""
    g = {k: np.asarray(v) for k, v in inp.items()}
    ids = g["input_ids"].astype(np.int64)
    embed = g["embed"].astype(np.float32)
    pos = np.arange(L, dtype=np.float32)[:, None]
    div = np.exp(np.arange(0, D, 2, dtype=np.float32) * (-math.log(10000.0) / D))
    ang = pos * div
    pe = np.zeros((L, D), dtype=np.float32)
    pe[:, 0::2] = np.sin(ang)
    pe[:, 1::2] = np.cos(ang)

    gam = g["gammas"].astype(np.float32)

    def tile_w(w):  # [Din, Dout] -> [P, Din//P, Dout]
        din, dout = w.shape
        return np.ascontiguousarray(
            w.reshape(din // P, P, dout).transpose(1, 0, 2)).astype(np.float32)

    wq_, wk_, wv_, wo_, wp_, wu_, wd_, vpat_ = [], [], [], [], [], [], [], []
    for i in range(n_layers):
        g1 = gam[2 * i][:, None]
        g2 = gam[2 * i + 1][:, None]
        wq_.append(tile_w(g1 * g["Wq"][i]))
        wk_.append(tile_w(g1 * g["Wk"][i]))
        wo_.append(tile_w(g["Wo"][i]))
        wp_.append(tile_w(g2 * g["Wproj"][i]))
        wu_.append(tile_w(g2 * g["Wup"][i]))
        wd_.append(tile_w(g["Wdown"][i]))
        wve = np.zeros((D, H, 65), np.float32)
        wve[:, :, 0:DH] = (g1 * g["Wv"][i]).reshape(D, H, DH)
        wv_.append(tile_w(wve.reshape(D, VROW)))
        vp = np.zeros((P, H, 65), np.float32)
        vp[0, :, 0:DH] = g["bv"][i].reshape(H, DH)
        vp[0, :, DH] = 1.0
        vpat_.append(vp.reshape(P, VROW))
    wq_, wk_, wv_, wo_ = (np.stack(a) for a in (wq_, wk_, wv_, wo_))
    wp_, wu_, wd_, vpat_ = (np.stack(a) for a in (wp_, wu_, wd_, vpat_))

    nz_bias = bool(
        any(np.any(g[k][:n_layers] != 0)
            for k in ("bq", "bk", "bo", "bproj", "bup", "bdown")))
    ball = None
    if nz_bias:
        ball = np.zeros((n_layers, P, 64), np.float32)
        for i in range(n_layers):
            ball[i, :, 0:8] = g["bq"][i].reshape(8, P).T
            ball[i, :, 8:16] = g["bk"][i].reshape(8, P).T
            ball[i, :, 16:24] = g["bo"][i].reshape(8, P).T
            ball[i, :, 24:40] = g["bproj"][i].reshape(16, P).T
            ball[i, :, 40:56] = g["bup"][i].reshape(16, P).T
            ball[i, :, 56:64] = g["bdown"][i].reshape(8, P).T
    nz_blog = do_logits and bool(np.any(g["blogits"] != 0))

    ones = np.ones((P, P), np.float32)
    e0 = np.zeros((P, P), np.float32)
    e0[0, :] = 1.0

    tri = np.tril(np.ones((P, P), np.float32)).T  # [k, q]: 1 if k <= q

    def to_bf16(a):
        import ml_dtypes
        return np.ascontiguousarray(a).astype(ml_dtypes.bfloat16)

    in_maps = []
    for c in range(NCORE):
        b, cb = c // 4, c % 4
        # interleaved ownership: local token j*128+r  <->  global (4j+cb)*128+r
        tok_idx = np.concatenate(
            [np.arange((4 * j + cb) * P, (4 * j + cb) * P + P) for j in range(NJ)])
        x0 = embed[ids[b, tok_idx]] + pe[tok_idx]
        x0T = np.ascontiguousarray(x0.T.reshape(KO, P, T).transpose(1, 0, 2))

        mk = np.zeros((NJ, P, P), np.float32)
        for i in range(NJ):
            mk[i] = 1.0 if i < cb else (tri if i == cb else 0.0)
        mk = mk.transpose(1, 0, 2)  # [P, NJ, P]

        m = {
            "x0T": x0T.astype(np.float32),
            "wq": to_bf16(wq_), "wk": to_bf16(wk_), "wv": to_bf16(wv_),
            "wo": to_bf16(wo_), "wproj": to_bf16(wp_), "wup": to_bf16(wu_),
            "wdown": to_bf16(wd_), "vpat": to_bf16(vpat_),
            "maskpat": to_bf16(mk), "ones_d": to_bf16(ones), "e0_d": to_bf16(e0),
        }
        if nz_bias:
            m["ball"] = ball
        if do_logits:
            wl = g["Wlogits"][:, c * VS:(c + 1) * VS].astype(np.float32)
            wlt = wl.reshape(KO, P, VT, VN).transpose(2, 1, 0, 3)  # [VT, P, KO, VN]
            m["wlog"] = to_bf16(wlt)
            if nz_blog:
                bl = np.zeros((P, VS), np.float32)
                bl[0, :] = g["blogits"][c * VS:(c + 1) * VS]
                m["blogpat"] = to_bf16(bl)
        in_maps.append(m)
    return in_maps, nz_bias, nz_blog


_CACHE = {}


def _get_nc(key):
    if key not in _CACHE:
        _CACHE[key] = build(n_layers=4, do_logits=True,
                            nz_bias=key[0], nz_blog=key[1])
    return _CACHE[key]


def kernel(**inputs):
    """Full-model entry: takes setup_inputs() arrays, returns [B, L, V] float32 logits."""
    from concourse.bass_utils import run_bass_kernel_spmd
    in_maps, nzb, nzbl = host_inputs(inputs, n_layers=4, do_logits=True)
    nc = _get_nc((nzb, nzbl))
    res = run_bass_kernel_spmd(nc, in_maps, core_ids=list(range(NCORE)))
    # logits rows are in global token order already; concat vocab shards
    out = np.concatenate(
        [res.results[c]["logits"].astype(np.float32).reshape(B, L, VS)
         for c in range(NCORE)], axis=-1)
    return np.ascontiguousarray(out, dtype=np.float32)
